# revision 33
# baseline (speedup 1.0000x reference)
import sys

sys.path.insert(0, "/opt/trn_rl_repo")
from contextlib import ExitStack

import numpy as np
import ml_dtypes

from concourse import bass, bacc, tile
from concourse.bass_utils import run_bass_kernel_spmd
from concourse.masks import make_identity

mybir = bass.mybir
AF = mybir.ActivationFunctionType
ALU = mybir.AluOpType
F32 = mybir.dt.float32
BF16 = mybir.dt.bfloat16
F16 = mybir.dt.float16
U32 = mybir.dt.uint32
U16 = mybir.dt.uint16
I16 = mybir.dt.int16
BF16NP = ml_dtypes.bfloat16

B = 8
N = 8192
S = 2048
DF = 256
DL = 128
O0 = 256
O1 = 128
NCHUNK = N // 128          # 64
GRP = 2                    # chunks per gather group (768 SWDGE descs/gather)
NGRP = NCHUNK // GRP       # 32
NI = GRP * 3               # gathered rows per point per group
NSLICE = N // 512          # 16
EPS_W = 1e-8
EPS_BN = 1e-5
INV_TOT = 1.0 / (B * N)

_CACHE = {}


# ---------------- host-side packing ----------------

def _split3(v):
    a = v.astype(BF16NP).astype(np.float32)
    r1 = v - a
    b = r1.astype(BF16NP).astype(np.float32)
    r2 = r1 - b
    c = r2.astype(BF16NP).astype(np.float32)
    return a, b, c


def _morton_order(xyz):
    # xyz: [S, 3] float32 -> permutation ordering points along a z-curve
    q = xyz - xyz.min(0, keepdims=True)
    q = q / (q.max(0, keepdims=True) + 1e-9)
    g = np.minimum((q * 1024).astype(np.int64), 1023)  # 10 bits per dim

    def spread(x):
        x = (x | (x << 16)) & 0x030000FF
        x = (x | (x << 8)) & 0x0300F00F
        x = (x | (x << 4)) & 0x030C30C3
        x = (x | (x << 2)) & 0x09249249
        return x

    code = (spread(g[:, 0]) << 2) | (spread(g[:, 1]) << 1) | spread(g[:, 2])
    return np.argsort(code, kind="stable")


def _pack_core(xyzl, xyzf, featf, featl):
    """Build per-core input arrays. xyzl [N,3], xyzf [S,3], featf [S,DF], featl [N,DL]."""
    perm = _morton_order(xyzf)
    xyzf = xyzf[perm]
    featf = featf[perm]

    # --- lhsT_all [24, 32, 256] bf16: large-point side ---
    # p_sb[c*32+blk, j] = xyzl[blk*256+j, c]; p2 = 2*p
    p = xyzl.astype(np.float32)           # [N, 3]
    p2 = 2.0 * p
    pa, pb, pc = _split3(p2)              # [N, 3] each
    pn2 = (p * p).sum(1)                  # [N]
    pna, pnb, pnc = _split3(pn2)

    lhsT = np.zeros((24, N), np.float32)
    # rows: groups of 3 coords: (pa, pa, pb, pa, pc, pb)
    for g, src in enumerate([pa, pa, pb, pa, pc, pb]):
        lhsT[3 * g:3 * g + 3, :] = src.T
    lhsT[18:21, :] = -1.0
    lhsT[21, :] = pna
    lhsT[22, :] = pnb
    lhsT[23, :] = pnc
    # reorder cols: [24, N] -> [24, 32, 256] with point p = blk*256 + j
    lhsT_all = lhsT.reshape(24, 32, 256).astype(BF16NP)

    # --- rhs_all [24, 2048] bf16: few-point side ---
    q = xyzf.astype(np.float32)
    qa, qb, qc = _split3(q)
    qn2 = (q * q).sum(1)
    qna, qnb, qnc = _split3(qn2)
    rhs = np.zeros((24, S), np.float32)
    for g, src in enumerate([qa, qb, qa, qc, qa, qb]):
        rhs[3 * g:3 * g + 3, :] = src.T
    rhs[18, :] = qna
    rhs[19, :] = qnb
    rhs[20, :] = qnc
    rhs[21:24, :] = -1.0
    rhs_all = rhs.astype(BF16NP)

    return {
        "lhsT": np.ascontiguousarray(lhsT_all),
        "rhs": np.ascontiguousarray(rhs_all),
        "featf": np.ascontiguousarray(featf.astype(BF16NP)),
        "featl": np.ascontiguousarray(featl.astype(BF16NP)),
    }


# ---------------- device kernel ----------------

def _build():
    nc = bacc.Bacc("TRN2", target_bir_lowering=False, debug=False, num_devices=B)

    lhsT_h = nc.dram_tensor("lhsT", [24, 32, 256], BF16, kind="ExternalInput")
    rhs_h = nc.dram_tensor("rhs", [24, S], BF16, kind="ExternalInput")
    featf_h = nc.dram_tensor("featf", [S, DF], BF16, kind="ExternalInput")
    featl_h = nc.dram_tensor("featl", [N, DL], BF16, kind="ExternalInput")
    rep_h = nc.dram_tensor("rep", [8, 128, 128], F16, kind="ExternalInput")
    w0t_h = nc.dram_tensor("w0t", [384, O0], BF16, kind="ExternalInput")
    w1t_h = nc.dram_tensor("w1t", [O0, O1], BF16, kind="ExternalInput")
    g0_h = nc.dram_tensor("g0", [O0], F32, kind="ExternalInput")
    bt0_h = nc.dram_tensor("bt0", [O0], F32, kind="ExternalInput")
    g1_h = nc.dram_tensor("g1", [O1], F32, kind="ExternalInput")
    bt1_h = nc.dram_tensor("bt1", [O1], F32, kind="ExternalInput")
    out_h = nc.dram_tensor("out", [O1, N], F32, kind="ExternalOutput")

    with tile.TileContext(nc) as tc:
        with ExitStack() as ctx:
            sb = ctx.enter_context(tc.tile_pool(name="sb", bufs=1))
            sb2 = ctx.enter_context(tc.tile_pool(name="sb2", bufs=2))
            sb3 = ctx.enter_context(tc.tile_pool(name="sb3", bufs=2))
            ps_d = ctx.enter_context(tc.tile_pool(name="psd", bufs=1, space="PSUM"))
            ps_t = ctx.enter_context(tc.tile_pool(name="pst", bufs=1, space="PSUM"))
            ps_y = ctx.enter_context(tc.tile_pool(name="psy", bufs=2, space="PSUM"))
            ps_g = ctx.enter_context(tc.tile_pool(name="psg", bufs=1, space="PSUM"))
            dr = ctx.enter_context(tc.tile_pool(name="dr", bufs=1, space="DRAM"))

            ident = sb.tile([128, 128], BF16)
            make_identity(nc, ident[:, :])
            epsb = sb.tile([128, 1], F32)
            nc.vector.memset(epsb[:, :], EPS_BN)
            h1024 = sb.tile([128, 16], U32)
            nc.vector.memset(h1024[:, 0:8], 0)
            nc.vector.memset(h1024[:, 8:16], 1024)
            rep8t = sb.tile([128, 8, 128], F16)
            nc.sync.dma_start(
                rep8t[:, :, :], bass.AP(rep_h, 0, [[128, 128], [128 * 128, 8], [1, 128]])
            )

            # ---- static loads ----
            lhsT_all = sb.tile([24, 32, 256], BF16)
            nc.sync.dma_start(lhsT_all[:, :, :], lhsT_h.ap())
            rhs_all = sb.tile([24, S], BF16)
            nc.sync.dma_start(rhs_all[:, :], rhs_h.ap())
            w0t = sb.tile([128, 3, O0], BF16)
            nc.sync.dma_start(w0t[:, :, :], bass.AP(w0t_h, 0, [[256, 128], [32768, 3], [1, 256]]))
            w1t = sb.tile([128, 2, O1], BF16)
            nc.sync.dma_start(w1t[:, :, :], bass.AP(w1t_h, 0, [[128, 128], [16384, 2], [1, 128]]))
            g0b = sb.tile([128, 2], F32)
            nc.sync.dma_start(g0b[:, :], bass.AP(g0_h, 0, [[1, 128], [128, 2]]))
            b0b = sb.tile([128, 2], F32)
            nc.sync.dma_start(b0b[:, :], bass.AP(bt0_h, 0, [[1, 128], [128, 2]]))
            g1b = sb.tile([128, 1], F32)
            nc.sync.dma_start(g1b[:, :], g1_h.ap())
            b1b = sb.tile([128, 1], F32)
            nc.sync.dma_start(b1b[:, :], bt1_h.ap())

            # ---- persistent activations + stats ----
            y0a = sb.tile([128, N], BF16)
            y0b = sb.tile([128, N], BF16)
            y1 = sb.tile([128, N], F32)
            s0a = sb.tile([128, NSLICE], F32)
            s0b = sb.tile([128, NSLICE], F32)
            q0a = sb.tile([128, NSLICE], F32)
            q0b = sb.tile([128, NSLICE], F32)
            s1 = sb.tile([128, NSLICE], F32)
            q1 = sb.tile([128, NSLICE], F32)

            featf_ap = featf_h.ap()

            # ---- main loop: gather-groups of 2 chunks, MLP-groups of 4 ----
            # Software-pipelined: group g's gather is consumed while group g+1
            # runs its distance/max scans, so DVE never stalls on gather DMA.
            pending = None  # (g, gf, w3acc) awaiting interp/transpose/MLP
            xT = None

            def consume_chunk(pg, t, gf, w3acc):
                nonlocal xT
                i = pg * GRP + t
                tq = i % 4
                # interp = sum_k w3[:, k] * gf[:, 3t+k, :]  (bf16 stt chain)
                itp = sb2.tile([128, DF], BF16, tag="itp", name="itp")
                nc.scalar.activation(itp[:, :], gf[:, 3 * t, :], AF.Copy, scale=w3acc[:, t, 0:1])
                for k in (1, 2):
                    nc.vector.scalar_tensor_tensor(
                        itp[:, :], gf[:, 3 * t + k, :], w3acc[:, t, k:k + 1], itp[:, :],
                        ALU.mult, ALU.add,
                    )

                fl = sb2.tile([128, DL], BF16, tag="fl", name="fl")
                nc.sync.dma_start(
                    fl[:, :], bass.AP(featl_h, i * 128 * DL, [[DL, 128], [1, DL]])
                )
                if tq == 0:
                    xT = [sb3.tile([128, 512], BF16, tag=f"xt{c}", name=f"xt{c}") for c in range(3)]
                ptx = ps_t.tile([128, 384], BF16, tag="pt", name="ptx")
                nc.tensor.transpose(ptx[:, 0:128], fl[:, :], ident[:, :])
                nc.tensor.transpose(ptx[:, 128:256], itp[:, 0:128], ident[:, :])
                nc.tensor.transpose(ptx[:, 256:384], itp[:, 128:256], ident[:, :])
                for c in range(3):
                    nc.scalar.activation(
                        xT[c][:, tq * 128:(tq + 1) * 128], ptx[:, c * 128:(c + 1) * 128], AF.Copy
                    )

                # MLP0 per 4 chunks (512 points)
                if tq == 3:
                    jn = i // 4
                    for ob in range(2):
                        py = ps_y.tile([128, 512], F32, tag="py", name="py")
                        for cb in range(3):
                            nc.tensor.matmul(
                                py[:, :],
                                w0t[:, cb, ob * 128:(ob + 1) * 128],
                                xT[cb][:, :],
                                start=(cb == 0), stop=(cb == 2),
                            )
                        y0t = y0a if ob == 0 else y0b
                        s0t = s0a if ob == 0 else s0b
                        q0t = q0a if ob == 0 else q0b
                        nc.scalar.activation(
                            y0t[:, jn * 512:(jn + 1) * 512], py[:, :], AF.Copy,
                            accum_out=s0t[:, jn:jn + 1],
                        )
                        trash = sb2.tile([128, 512], F32, tag="trash", name="trash")
                        nc.scalar.activation(
                            trash[:, :], py[:, :], AF.Square,
                            accum_out=q0t[:, jn:jn + 1],
                        )

            for g in range(NGRP):
                idxacc = sb3.tile([128, GRP, 8], U32, tag="idxacc", name="idxacc")
                w3acc = sb3.tile([128, GRP, 3], F32, tag="w3acc", name="w3acc")
                for t in range(GRP):
                    i = g * GRP + t
                    blk, half = divmod(i, 2)
                    lhs_chunk = lhsT_all[:, blk, half * 128:half * 128 + 128]

                    psd = ps_d.tile([128, S], F32, tag="psd", name="psd")
                    for m in range(4):
                        nc.tensor.matmul(
                            psd[:, m * 512:(m + 1) * 512], lhs_chunk,
                            rhs_all[:, m * 512:(m + 1) * 512],
                            start=True, stop=True,
                        )

                    if t == 0:
                        maxacc = sb2.tile([128, GRP, 8], F32, tag="maxacc", name="maxacc")
                    nc.vector.max(maxacc[:, t, :], psd[:, :])
                    nc.vector.max_index(idxacc[:, t, :], maxacc[:, t, :], psd[:, :])

                    # fill the dist(t+1)-wait gap with the previous group's
                    # interp/transpose/MLP work
                    if pending is not None:
                        consume_chunk(pending[0], t, pending[1], pending[2])

                # weights (batched per group): w3 = normalize(1 / (eps - maxv3))
                d3 = sb2.tile([128, GRP, 3], F32, tag="d3", name="d3")
                nc.scalar.activation(d3[:, :, :], maxacc[:, :, 0:3], AF.Copy, bias=EPS_W, scale=-1.0)
                rec = sb2.tile([128, GRP, 3], F32, tag="rec", name="rec")
                nc.vector.reciprocal(rec[:, :, :], d3[:, :, :])
                rsi = sb2.tile([128, GRP], F32, tag="rsi", name="rsi")
                for t in range(GRP):
                    nc.vector.tensor_reduce(rsi[:, t:t + 1], rec[:, t, :], mybir.AxisListType.X, ALU.add)
                nc.vector.reciprocal(rsi[:, :], rsi[:, :])
                for t in range(GRP):
                    nc.vector.scalar_tensor_tensor(
                        w3acc[:, t, :], rec[:, t, :], rsi[:, t:t + 1], rec[:, t, :],
                        ALU.mult, ALU.bypass,
                    )

                # gather-index table T[16c+q, 8i+c'] = idx[16c'+q, i], replicated
                idxh = sb2.tile([128, NI], F16, tag="idxh", name="idxh")
                for t in range(GRP):
                    nc.vector.tensor_copy(idxh[:, 3 * t:3 * t + 3], idxacc[:, t, 0:3])
                tps = ps_g.tile([128, 8 * NI], F32, tag="tps", name="tps")
                for c in range(8):
                    nc.tensor.matmul(
                        bass.AP(tps.tensor, c, [[8 * NI, 128], [8, NI]]),
                        rep8t[:, c, :], idxh[:, :],
                        start=True, stop=True,
                    )
                tbl = sb2.tile([128, 8 * NI], U16, tag="tbl", name="tbl")
                nc.vector.tensor_copy(tbl[:, :], tps[:, :])
                gf = sb3.tile([128, NI, DF], BF16, tag="gf", name="gf")
                nc.gpsimd.dma_gather(
                    gf[:, :, :], featf_ap, tbl[:, :].bitcast(I16), 128 * NI, 128 * NI, DF
                )

                pending = (g, gf, w3acc)
            for t in range(GRP):
                consume_chunk(pending[0], t, pending[1], pending[2])

            # ---- BN0 stats (cross-device) ----
            st0 = sb.tile([128, 4], F32)
            nc.vector.tensor_reduce(st0[:, 0:1], s0a[:, :], mybir.AxisListType.X, ALU.add)
            nc.vector.tensor_reduce(st0[:, 1:2], s0b[:, :], mybir.AxisListType.X, ALU.add)
            nc.vector.tensor_reduce(st0[:, 2:3], q0a[:, :], mybir.AxisListType.X, ALU.add)
            nc.vector.tensor_reduce(st0[:, 3:4], q0b[:, :], mybir.AxisListType.X, ALU.add)
            cin0 = dr.tile([128, 4], F32)
            cout0 = dr.tile([8, 512], F32, addr_space="Shared")
            nc.gpsimd.dma_start(cin0[:, :], st0[:, :])
            nc.gpsimd.collective_compute(
                "AllGather", ALU.bypass,
                replica_groups=[list(range(B))],
                ins=[cin0.opt()], outs=[cout0.opt()],
            )
            stg0g = sb.tile([128, 4, 8], F32)
            nc.gpsimd.dma_start(
                stg0g[:, :, :], bass.AP(cout0.tensor, 0, [[4, 128], [1, 4], [512, 8]])
            )
            stg0 = sb.tile([128, 4], F32)
            nc.vector.tensor_reduce(stg0[:, :], stg0g[:, :, :], mybir.AxisListType.X, ALU.add)

            mn0 = sb.tile([128, 4], F32)
            nc.scalar.activation(mn0[:, :], stg0[:, :], AF.Copy, scale=INV_TOT)
            var0 = sb.tile([128, 2], F32)
            nc.vector.tensor_tensor(var0[:, :], mn0[:, 0:2], mn0[:, 0:2], ALU.mult)
            nc.vector.tensor_tensor(var0[:, :], mn0[:, 2:4], var0[:, :], ALU.subtract)
            std0 = sb.tile([128, 2], F32)
            nc.scalar.activation(std0[:, :], var0[:, :], AF.Sqrt, bias=epsb[:, 0:1])
            rstd0 = sb.tile([128, 2], F32)
            nc.vector.reciprocal(rstd0[:, :], std0[:, :])
            a0 = sb.tile([128, 2], F32)
            nc.vector.tensor_tensor(a0[:, :], g0b[:, :], rstd0[:, :], ALU.mult)
            c0 = sb.tile([128, 2], F32)
            nc.vector.tensor_tensor(c0[:, :], mn0[:, 0:2], a0[:, :], ALU.mult)
            nc.vector.tensor_tensor(c0[:, :], b0b[:, :], c0[:, :], ALU.subtract)

            # ---- BN0 apply + layer 1, pipelined per 512-slice ----
            for jn in range(NSLICE):
                sl = slice(jn * 512, (jn + 1) * 512)
                # ob=0 on DVE (two fused tensor_scalar ops), ob=1 on Act
                nc.vector.tensor_scalar(
                    out=y0a[:, sl], in0=y0a[:, sl],
                    scalar1=a0[:, 0:1], scalar2=c0[:, 0:1],
                    op0=ALU.mult, op1=ALU.add,
                )
                nc.vector.tensor_scalar(
                    out=y0a[:, sl], in0=y0a[:, sl],
                    scalar1=0.0, scalar2=0.0,
                    op0=ALU.max, op1=ALU.bypass,
                )
                nc.scalar.activation(
                    y0b[:, sl], y0b[:, sl],
                    AF.Relu, bias=c0[:, 1:2], scale=a0[:, 1:2],
                )
                py = ps_y.tile([128, 512], F32, tag="py", name="py1")
                nc.tensor.matmul(
                    py[:, :], w1t[:, 0, :],
                    y0a[:, jn * 512:(jn + 1) * 512],
                    start=True, stop=False,
                )
                nc.tensor.matmul(
                    py[:, :], w1t[:, 1, :],
                    y0b[:, jn * 512:(jn + 1) * 512],
                    start=False, stop=True,
                )
                nc.scalar.activation(
                    y1[:, jn * 512:(jn + 1) * 512], py[:, :], AF.Copy,
                    accum_out=s1[:, jn:jn + 1],
                )
                trash = sb2.tile([128, 512], F32, tag="trash", name="trash1")
                nc.scalar.activation(
                    trash[:, :], py[:, :], AF.Square, accum_out=q1[:, jn:jn + 1],
                )

            # ---- BN1 ----
            st1 = sb.tile([128, 2], F32)
            nc.vector.tensor_reduce(st1[:, 0:1], s1[:, :], mybir.AxisListType.X, ALU.add)
            nc.vector.tensor_reduce(st1[:, 1:2], q1[:, :], mybir.AxisListType.X, ALU.add)
            cin1 = dr.tile([128, 2], F32)
            cout1 = dr.tile([8, 256], F32, addr_space="Shared")
            nc.gpsimd.dma_start(cin1[:, :], st1[:, :])
            nc.gpsimd.collective_compute(
                "AllGather", ALU.bypass,
                replica_groups=[list(range(B))],
                ins=[cin1.opt()], outs=[cout1.opt()],
            )
            stg1g = sb.tile([128, 2, 8], F32)
            nc.gpsimd.dma_start(
                stg1g[:, :, :], bass.AP(cout1.tensor, 0, [[2, 128], [1, 2], [256, 8]])
            )
            stg1 = sb.tile([128, 2], F32)
            nc.vector.tensor_reduce(stg1[:, :], stg1g[:, :, :], mybir.AxisListType.X, ALU.add)

            mn1 = sb.tile([128, 2], F32)
            nc.scalar.activation(mn1[:, :], stg1[:, :], AF.Copy, scale=INV_TOT)
            var1 = sb.tile([128, 1], F32)
            nc.vector.tensor_tensor(var1[:, :], mn1[:, 0:1], mn1[:, 0:1], ALU.mult)
            nc.vector.tensor_tensor(var1[:, :], mn1[:, 1:2], var1[:, :], ALU.subtract)
            std1 = sb.tile([128, 1], F32)
            nc.scalar.activation(std1[:, :], var1[:, :], AF.Sqrt, bias=epsb[:, 0:1])
            rstd1 = sb.tile([128, 1], F32)
            nc.vector.reciprocal(rstd1[:, :], std1[:, :])
            a1 = sb.tile([128, 1], F32)
            nc.vector.tensor_tensor(a1[:, :], g1b[:, :], rstd1[:, :], ALU.mult)
            c1 = sb.tile([128, 1], F32)
            nc.vector.tensor_tensor(c1[:, :], mn1[:, 0:1], a1[:, :], ALU.mult)
            nc.vector.tensor_tensor(c1[:, :], b1b[:, :], c1[:, :], ALU.subtract)
            for sl in range(4):
                nc.scalar.activation(
                    y1[:, sl * 2048:(sl + 1) * 2048], y1[:, sl * 2048:(sl + 1) * 2048],
                    AF.Relu, bias=c1[:, 0:1], scale=a1[:, 0:1],
                )
                nc.sync.dma_start(
                    bass.AP(out_h, sl * 2048, [[N, 128], [1, 2048]]),
                    y1[:, sl * 2048:(sl + 1) * 2048],
                )
    nc.compile()
    return nc


def kernel(**inputs):
    xyzl = np.asarray(inputs["point_xyz_large"], dtype=np.float32)
    xyzf = np.asarray(inputs["point_xyz_few"], dtype=np.float32)
    featf = np.asarray(inputs["point_feature_few"], dtype=np.float32)
    featl = np.asarray(inputs["point_feature_large"], dtype=np.float32)
    w0 = np.asarray(inputs["W0"], dtype=np.float32)
    w1 = np.asarray(inputs["W1"], dtype=np.float32)
    g0 = np.asarray(inputs["g0"], dtype=np.float32)
    bt0 = np.asarray(inputs["beta0"], dtype=np.float32)
    g1 = np.asarray(inputs["g1"], dtype=np.float32)
    bt1 = np.asarray(inputs["beta1"], dtype=np.float32)

    w0t = np.ascontiguousarray(w0.T.astype(BF16NP))   # [384, 256]
    w1t = np.ascontiguousarray(w1.T.astype(BF16NP))   # [256, 128]

    if "rep" not in _CACHE:
        rep = np.zeros((8, 128, 128), np.float16)
        for c in range(8):
            for m in range(128):
                rep[c, 16 * c + m % 16, m] = 1.0
        _CACHE["rep"] = rep
    rep = _CACHE["rep"]

    if "nc" not in _CACHE:
        _CACHE["nc"] = _build()
    nc = _CACHE["nc"]

    in_maps = []
    for b in range(B):
        m = _pack_core(xyzl[b], xyzf[b], featf[b], featl[b])
        m.update({
            "rep": rep, "w0t": w0t, "w1t": w1t,
            "g0": g0, "bt0": bt0, "g1": g1, "bt1": bt1,
        })
        in_maps.append(m)

    res = run_bass_kernel_spmd(
        nc, in_maps, list(range(B)), trace=_CACHE.get("trace", False)
    )
    out = np.stack([np.asarray(res.results[b]["out"]) for b in range(B)], 0)
    _CACHE["last_res"] = res
    return out.astype(np.float32)


# revision 34
# speedup vs baseline: 1.0217x; 1.0217x over previous
import sys

sys.path.insert(0, "/opt/trn_rl_repo")
from contextlib import ExitStack

import numpy as np
import ml_dtypes

from concourse import bass, bacc, tile
from concourse.bass_utils import run_bass_kernel_spmd
from concourse.masks import make_identity

mybir = bass.mybir
AF = mybir.ActivationFunctionType
ALU = mybir.AluOpType
F32 = mybir.dt.float32
BF16 = mybir.dt.bfloat16
F16 = mybir.dt.float16
U32 = mybir.dt.uint32
U16 = mybir.dt.uint16
I16 = mybir.dt.int16
BF16NP = ml_dtypes.bfloat16

B = 8
N = 8192
S = 2048
DF = 256
DL = 128
O0 = 256
O1 = 128
NCHUNK = N // 128          # 64
GRP = 4                    # chunks per gather group (2 x 768-desc gathers)
NGRP = NCHUNK // GRP       # 16
NI = GRP * 3               # gathered rows per point per group
NSLICE = N // 512          # 16
EPS_W = 1e-8
EPS_BN = 1e-5
INV_TOT = 1.0 / (B * N)

_CACHE = {}


# ---------------- host-side packing ----------------

def _split3(v):
    a = v.astype(BF16NP).astype(np.float32)
    r1 = v - a
    b = r1.astype(BF16NP).astype(np.float32)
    r2 = r1 - b
    c = r2.astype(BF16NP).astype(np.float32)
    return a, b, c


def _morton_order(xyz):
    # xyz: [S, 3] float32 -> permutation ordering points along a z-curve
    q = xyz - xyz.min(0, keepdims=True)
    q = q / (q.max(0, keepdims=True) + 1e-9)
    g = np.minimum((q * 1024).astype(np.int64), 1023)  # 10 bits per dim

    def spread(x):
        x = (x | (x << 16)) & 0x030000FF
        x = (x | (x << 8)) & 0x0300F00F
        x = (x | (x << 4)) & 0x030C30C3
        x = (x | (x << 2)) & 0x09249249
        return x

    code = (spread(g[:, 0]) << 2) | (spread(g[:, 1]) << 1) | spread(g[:, 2])
    return np.argsort(code, kind="stable")


def _pack_core(xyzl, xyzf, featf, featl):
    """Build per-core input arrays. xyzl [N,3], xyzf [S,3], featf [S,DF], featl [N,DL]."""
    perm = _morton_order(xyzf)
    xyzf = xyzf[perm]
    featf = featf[perm]

    # --- lhsT_all [24, 32, 256] bf16: large-point side ---
    # p_sb[c*32+blk, j] = xyzl[blk*256+j, c]; p2 = 2*p
    p = xyzl.astype(np.float32)           # [N, 3]
    p2 = 2.0 * p
    pa, pb, pc = _split3(p2)              # [N, 3] each
    pn2 = (p * p).sum(1)                  # [N]
    pna, pnb, pnc = _split3(pn2)

    lhsT = np.zeros((24, N), np.float32)
    # rows: groups of 3 coords: (pa, pa, pb, pa, pc, pb)
    for g, src in enumerate([pa, pa, pb, pa, pc, pb]):
        lhsT[3 * g:3 * g + 3, :] = src.T
    lhsT[18:21, :] = -1.0
    lhsT[21, :] = pna
    lhsT[22, :] = pnb
    lhsT[23, :] = pnc
    # reorder cols: [24, N] -> [24, 32, 256] with point p = blk*256 + j
    lhsT_all = lhsT.reshape(24, 32, 256).astype(BF16NP)

    # --- rhs_all [24, 2048] bf16: few-point side ---
    q = xyzf.astype(np.float32)
    qa, qb, qc = _split3(q)
    qn2 = (q * q).sum(1)
    qna, qnb, qnc = _split3(qn2)
    rhs = np.zeros((24, S), np.float32)
    for g, src in enumerate([qa, qb, qa, qc, qa, qb]):
        rhs[3 * g:3 * g + 3, :] = src.T
    rhs[18, :] = qna
    rhs[19, :] = qnb
    rhs[20, :] = qnc
    rhs[21:24, :] = -1.0
    rhs_all = rhs.astype(BF16NP)

    return {
        "lhsT": np.ascontiguousarray(lhsT_all),
        "rhs": np.ascontiguousarray(rhs_all),
        "featf": np.ascontiguousarray(featf.astype(BF16NP)),
        "featl": np.ascontiguousarray(featl.astype(BF16NP)),
    }


# ---------------- device kernel ----------------

def _build():
    nc = bacc.Bacc("TRN2", target_bir_lowering=False, debug=False, num_devices=B)

    lhsT_h = nc.dram_tensor("lhsT", [24, 32, 256], BF16, kind="ExternalInput")
    rhs_h = nc.dram_tensor("rhs", [24, S], BF16, kind="ExternalInput")
    featf_h = nc.dram_tensor("featf", [S, DF], BF16, kind="ExternalInput")
    featl_h = nc.dram_tensor("featl", [N, DL], BF16, kind="ExternalInput")
    rep_h = nc.dram_tensor("rep", [8, 128, 128], F16, kind="ExternalInput")
    w0t_h = nc.dram_tensor("w0t", [384, O0], BF16, kind="ExternalInput")
    w1t_h = nc.dram_tensor("w1t", [O0, O1], BF16, kind="ExternalInput")
    g0_h = nc.dram_tensor("g0", [O0], F32, kind="ExternalInput")
    bt0_h = nc.dram_tensor("bt0", [O0], F32, kind="ExternalInput")
    g1_h = nc.dram_tensor("g1", [O1], F32, kind="ExternalInput")
    bt1_h = nc.dram_tensor("bt1", [O1], F32, kind="ExternalInput")
    out_h = nc.dram_tensor("out", [O1, N], F32, kind="ExternalOutput")

    with tile.TileContext(nc) as tc:
        with ExitStack() as ctx:
            sb = ctx.enter_context(tc.tile_pool(name="sb", bufs=1))
            sb2 = ctx.enter_context(tc.tile_pool(name="sb2", bufs=2))
            sb3 = ctx.enter_context(tc.tile_pool(name="sb3", bufs=2))
            ps_d = ctx.enter_context(tc.tile_pool(name="psd", bufs=1, space="PSUM"))
            ps_t = ctx.enter_context(tc.tile_pool(name="pst", bufs=1, space="PSUM"))
            ps_y = ctx.enter_context(tc.tile_pool(name="psy", bufs=2, space="PSUM"))
            ps_g = ctx.enter_context(tc.tile_pool(name="psg", bufs=1, space="PSUM"))
            dr = ctx.enter_context(tc.tile_pool(name="dr", bufs=1, space="DRAM"))

            ident = sb.tile([128, 128], BF16)
            make_identity(nc, ident[:, :])
            epsb = sb.tile([128, 1], F32)
            nc.vector.memset(epsb[:, :], EPS_BN)
            h1024 = sb.tile([128, 16], U32)
            nc.vector.memset(h1024[:, 0:8], 0)
            nc.vector.memset(h1024[:, 8:16], 1024)
            rep8t = sb.tile([128, 8, 128], F16)
            nc.sync.dma_start(
                rep8t[:, :, :], bass.AP(rep_h, 0, [[128, 128], [128 * 128, 8], [1, 128]])
            )

            # ---- static loads ----
            lhsT_all = sb.tile([24, 32, 256], BF16)
            nc.sync.dma_start(lhsT_all[:, :, :], lhsT_h.ap())
            rhs_all = sb.tile([24, S], BF16)
            nc.sync.dma_start(rhs_all[:, :], rhs_h.ap())
            w0t = sb.tile([128, 3, O0], BF16)
            nc.sync.dma_start(w0t[:, :, :], bass.AP(w0t_h, 0, [[256, 128], [32768, 3], [1, 256]]))
            w1t = sb.tile([128, 2, O1], BF16)
            nc.sync.dma_start(w1t[:, :, :], bass.AP(w1t_h, 0, [[128, 128], [16384, 2], [1, 128]]))
            g0b = sb.tile([128, 2], F32)
            nc.sync.dma_start(g0b[:, :], bass.AP(g0_h, 0, [[1, 128], [128, 2]]))
            b0b = sb.tile([128, 2], F32)
            nc.sync.dma_start(b0b[:, :], bass.AP(bt0_h, 0, [[1, 128], [128, 2]]))
            g1b = sb.tile([128, 1], F32)
            nc.sync.dma_start(g1b[:, :], g1_h.ap())
            b1b = sb.tile([128, 1], F32)
            nc.sync.dma_start(b1b[:, :], bt1_h.ap())

            # ---- persistent activations + stats ----
            y0a = sb.tile([128, N], BF16)
            y0b = sb.tile([128, N], BF16)
            y1 = sb.tile([128, N], F32)
            s0a = sb.tile([128, NSLICE], F32)
            s0b = sb.tile([128, NSLICE], F32)
            q0a = sb.tile([128, NSLICE], F32)
            q0b = sb.tile([128, NSLICE], F32)
            s1 = sb.tile([128, NSLICE], F32)
            q1 = sb.tile([128, NSLICE], F32)

            featf_ap = featf_h.ap()

            # ---- main loop: gather-groups of 2 chunks, MLP-groups of 4 ----
            # Software-pipelined: group g's gather is consumed while group g+1
            # runs its distance/max scans, so DVE never stalls on gather DMA.
            pending = None  # (g, gf, w3acc) awaiting interp/transpose/MLP
            xT = None

            def consume_chunk(pg, t, gf, w3acc):
                nonlocal xT
                i = pg * GRP + t
                tq = i % 4
                # interp = sum_k w3[:, k] * gf[:, 3t+k, :]  (bf16 stt chain)
                itp = sb2.tile([128, DF], BF16, tag="itp", name="itp")
                nc.scalar.activation(itp[:, :], gf[:, 3 * t, :], AF.Copy, scale=w3acc[:, t, 0:1])
                for k in (1, 2):
                    nc.vector.scalar_tensor_tensor(
                        itp[:, :], gf[:, 3 * t + k, :], w3acc[:, t, k:k + 1], itp[:, :],
                        ALU.mult, ALU.add,
                    )

                fl = sb2.tile([128, DL], BF16, tag="fl", name="fl")
                nc.sync.dma_start(
                    fl[:, :], bass.AP(featl_h, i * 128 * DL, [[DL, 128], [1, DL]])
                )
                if tq == 0:
                    xT = [sb3.tile([128, 512], BF16, tag=f"xt{c}", name=f"xt{c}") for c in range(3)]
                ptx = ps_t.tile([128, 384], BF16, tag="pt", name="ptx")
                nc.tensor.transpose(ptx[:, 0:128], fl[:, :], ident[:, :])
                nc.tensor.transpose(ptx[:, 128:256], itp[:, 0:128], ident[:, :])
                nc.tensor.transpose(ptx[:, 256:384], itp[:, 128:256], ident[:, :])
                for c in range(3):
                    nc.scalar.activation(
                        xT[c][:, tq * 128:(tq + 1) * 128], ptx[:, c * 128:(c + 1) * 128], AF.Copy
                    )

                # MLP0 per 4 chunks (512 points)
                if tq == 3:
                    jn = i // 4
                    for ob in range(2):
                        py = ps_y.tile([128, 512], F32, tag="py", name="py")
                        for cb in range(3):
                            nc.tensor.matmul(
                                py[:, :],
                                w0t[:, cb, ob * 128:(ob + 1) * 128],
                                xT[cb][:, :],
                                start=(cb == 0), stop=(cb == 2),
                            )
                        y0t = y0a if ob == 0 else y0b
                        s0t = s0a if ob == 0 else s0b
                        q0t = q0a if ob == 0 else q0b
                        nc.scalar.activation(
                            y0t[:, jn * 512:(jn + 1) * 512], py[:, :], AF.Copy,
                            accum_out=s0t[:, jn:jn + 1],
                        )
                        trash = sb2.tile([128, 512], F32, tag="trash", name="trash")
                        nc.scalar.activation(
                            trash[:, :], py[:, :], AF.Square,
                            accum_out=q0t[:, jn:jn + 1],
                        )

            for g in range(NGRP):
                idxacc = sb3.tile([128, GRP, 8], U32, tag="idxacc", name="idxacc")
                w3acc = sb3.tile([128, GRP, 3], F32, tag="w3acc", name="w3acc")
                for t in range(GRP):
                    i = g * GRP + t
                    blk, half = divmod(i, 2)
                    lhs_chunk = lhsT_all[:, blk, half * 128:half * 128 + 128]

                    psd = ps_d.tile([128, S], F32, tag="psd", name="psd")
                    for m in range(4):
                        nc.tensor.matmul(
                            psd[:, m * 512:(m + 1) * 512], lhs_chunk,
                            rhs_all[:, m * 512:(m + 1) * 512],
                            start=True, stop=True,
                        )

                    if t == 0:
                        maxacc = sb2.tile([128, GRP, 8], F32, tag="maxacc", name="maxacc")
                    nc.vector.max(maxacc[:, t, :], psd[:, :])
                    nc.vector.max_index(idxacc[:, t, :], maxacc[:, t, :], psd[:, :])

                    # fill the dist(t+1)-wait gap with the previous group's
                    # interp/transpose/MLP work
                    if pending is not None:
                        consume_chunk(pending[0], t, pending[1], pending[2])

                # weights (batched per group): w3 = normalize(1 / (eps - maxv3))
                d3 = sb2.tile([128, GRP, 3], F32, tag="d3", name="d3")
                nc.scalar.activation(d3[:, :, :], maxacc[:, :, 0:3], AF.Copy, bias=EPS_W, scale=-1.0)
                rec = sb2.tile([128, GRP, 3], F32, tag="rec", name="rec")
                nc.vector.reciprocal(rec[:, :, :], d3[:, :, :])
                rsi = sb2.tile([128, GRP], F32, tag="rsi", name="rsi")
                for t in range(GRP):
                    nc.vector.tensor_reduce(rsi[:, t:t + 1], rec[:, t, :], mybir.AxisListType.X, ALU.add)
                nc.vector.reciprocal(rsi[:, :], rsi[:, :])
                for t in range(GRP):
                    nc.vector.scalar_tensor_tensor(
                        w3acc[:, t, :], rec[:, t, :], rsi[:, t:t + 1], rec[:, t, :],
                        ALU.mult, ALU.bypass,
                    )

                # gather-index table T[16c+q, 8i+c'] = idx[16c'+q, i], replicated
                idxh = sb2.tile([128, NI], F16, tag="idxh", name="idxh")
                for t in range(GRP):
                    nc.vector.tensor_copy(idxh[:, 3 * t:3 * t + 3], idxacc[:, t, 0:3])
                tps = ps_g.tile([128, 8 * NI], F32, tag="tps", name="tps")
                for c in range(8):
                    nc.tensor.matmul(
                        bass.AP(tps.tensor, c, [[8 * NI, 128], [8, NI]]),
                        rep8t[:, c, :], idxh[:, :],
                        start=True, stop=True,
                    )
                tbl = sb2.tile([128, 8 * NI], U16, tag="tbl", name="tbl")
                nc.vector.tensor_copy(tbl[:, :], tps[:, :])
                gf = sb3.tile([128, NI, DF], BF16, tag="gf", name="gf")
                for hg in range(2):
                    nh = NI // 2
                    nc.gpsimd.dma_gather(
                        gf[:, hg * nh:(hg + 1) * nh, :], featf_ap,
                        tbl[:, hg * 8 * nh:(hg + 1) * 8 * nh].bitcast(I16),
                        128 * nh, 128 * nh, DF,
                    )

                pending = (g, gf, w3acc)
            for t in range(GRP):
                consume_chunk(pending[0], t, pending[1], pending[2])

            # ---- BN0 stats (cross-device) ----
            st0 = sb.tile([128, 4], F32)
            nc.vector.tensor_reduce(st0[:, 0:1], s0a[:, :], mybir.AxisListType.X, ALU.add)
            nc.vector.tensor_reduce(st0[:, 1:2], s0b[:, :], mybir.AxisListType.X, ALU.add)
            nc.vector.tensor_reduce(st0[:, 2:3], q0a[:, :], mybir.AxisListType.X, ALU.add)
            nc.vector.tensor_reduce(st0[:, 3:4], q0b[:, :], mybir.AxisListType.X, ALU.add)
            cin0 = dr.tile([128, 4], F32)
            cout0 = dr.tile([8, 512], F32, addr_space="Shared")
            nc.gpsimd.dma_start(cin0[:, :], st0[:, :])
            nc.gpsimd.collective_compute(
                "AllGather", ALU.bypass,
                replica_groups=[list(range(B))],
                ins=[cin0.opt()], outs=[cout0.opt()],
            )
            stg0g = sb.tile([128, 4, 8], F32)
            nc.gpsimd.dma_start(
                stg0g[:, :, :], bass.AP(cout0.tensor, 0, [[4, 128], [1, 4], [512, 8]])
            )
            stg0 = sb.tile([128, 4], F32)
            nc.vector.tensor_reduce(stg0[:, :], stg0g[:, :, :], mybir.AxisListType.X, ALU.add)

            mn0 = sb.tile([128, 4], F32)
            nc.scalar.activation(mn0[:, :], stg0[:, :], AF.Copy, scale=INV_TOT)
            var0 = sb.tile([128, 2], F32)
            nc.vector.tensor_tensor(var0[:, :], mn0[:, 0:2], mn0[:, 0:2], ALU.mult)
            nc.vector.tensor_tensor(var0[:, :], mn0[:, 2:4], var0[:, :], ALU.subtract)
            std0 = sb.tile([128, 2], F32)
            nc.scalar.activation(std0[:, :], var0[:, :], AF.Sqrt, bias=epsb[:, 0:1])
            rstd0 = sb.tile([128, 2], F32)
            nc.vector.reciprocal(rstd0[:, :], std0[:, :])
            a0 = sb.tile([128, 2], F32)
            nc.vector.tensor_tensor(a0[:, :], g0b[:, :], rstd0[:, :], ALU.mult)
            c0 = sb.tile([128, 2], F32)
            nc.vector.tensor_tensor(c0[:, :], mn0[:, 0:2], a0[:, :], ALU.mult)
            nc.vector.tensor_tensor(c0[:, :], b0b[:, :], c0[:, :], ALU.subtract)

            # ---- BN0 apply + layer 1, pipelined per 512-slice ----
            for jn in range(NSLICE):
                sl = slice(jn * 512, (jn + 1) * 512)
                # ob=0 on DVE (two fused tensor_scalar ops), ob=1 on Act
                nc.vector.tensor_scalar(
                    out=y0a[:, sl], in0=y0a[:, sl],
                    scalar1=a0[:, 0:1], scalar2=c0[:, 0:1],
                    op0=ALU.mult, op1=ALU.add,
                )
                nc.vector.tensor_scalar(
                    out=y0a[:, sl], in0=y0a[:, sl],
                    scalar1=0.0, scalar2=0.0,
                    op0=ALU.max, op1=ALU.bypass,
                )
                nc.scalar.activation(
                    y0b[:, sl], y0b[:, sl],
                    AF.Relu, bias=c0[:, 1:2], scale=a0[:, 1:2],
                )
                py = ps_y.tile([128, 512], F32, tag="py", name="py1")
                nc.tensor.matmul(
                    py[:, :], w1t[:, 0, :],
                    y0a[:, jn * 512:(jn + 1) * 512],
                    start=True, stop=False,
                )
                nc.tensor.matmul(
                    py[:, :], w1t[:, 1, :],
                    y0b[:, jn * 512:(jn + 1) * 512],
                    start=False, stop=True,
                )
                nc.scalar.activation(
                    y1[:, jn * 512:(jn + 1) * 512], py[:, :], AF.Copy,
                    accum_out=s1[:, jn:jn + 1],
                )
                trash = sb2.tile([128, 512], F32, tag="trash", name="trash1")
                nc.scalar.activation(
                    trash[:, :], py[:, :], AF.Square, accum_out=q1[:, jn:jn + 1],
                )

            # ---- BN1 ----
            st1 = sb.tile([128, 2], F32)
            nc.vector.tensor_reduce(st1[:, 0:1], s1[:, :], mybir.AxisListType.X, ALU.add)
            nc.vector.tensor_reduce(st1[:, 1:2], q1[:, :], mybir.AxisListType.X, ALU.add)
            cin1 = dr.tile([128, 2], F32)
            cout1 = dr.tile([8, 256], F32, addr_space="Shared")
            nc.gpsimd.dma_start(cin1[:, :], st1[:, :])
            nc.gpsimd.collective_compute(
                "AllGather", ALU.bypass,
                replica_groups=[list(range(B))],
                ins=[cin1.opt()], outs=[cout1.opt()],
            )
            stg1g = sb.tile([128, 2, 8], F32)
            nc.gpsimd.dma_start(
                stg1g[:, :, :], bass.AP(cout1.tensor, 0, [[2, 128], [1, 2], [256, 8]])
            )
            stg1 = sb.tile([128, 2], F32)
            nc.vector.tensor_reduce(stg1[:, :], stg1g[:, :, :], mybir.AxisListType.X, ALU.add)

            mn1 = sb.tile([128, 2], F32)
            nc.scalar.activation(mn1[:, :], stg1[:, :], AF.Copy, scale=INV_TOT)
            var1 = sb.tile([128, 1], F32)
            nc.vector.tensor_tensor(var1[:, :], mn1[:, 0:1], mn1[:, 0:1], ALU.mult)
            nc.vector.tensor_tensor(var1[:, :], mn1[:, 1:2], var1[:, :], ALU.subtract)
            std1 = sb.tile([128, 1], F32)
            nc.scalar.activation(std1[:, :], var1[:, :], AF.Sqrt, bias=epsb[:, 0:1])
            rstd1 = sb.tile([128, 1], F32)
            nc.vector.reciprocal(rstd1[:, :], std1[:, :])
            a1 = sb.tile([128, 1], F32)
            nc.vector.tensor_tensor(a1[:, :], g1b[:, :], rstd1[:, :], ALU.mult)
            c1 = sb.tile([128, 1], F32)
            nc.vector.tensor_tensor(c1[:, :], mn1[:, 0:1], a1[:, :], ALU.mult)
            nc.vector.tensor_tensor(c1[:, :], b1b[:, :], c1[:, :], ALU.subtract)
            for sl in range(4):
                nc.scalar.activation(
                    y1[:, sl * 2048:(sl + 1) * 2048], y1[:, sl * 2048:(sl + 1) * 2048],
                    AF.Relu, bias=c1[:, 0:1], scale=a1[:, 0:1],
                )
                nc.sync.dma_start(
                    bass.AP(out_h, sl * 2048, [[N, 128], [1, 2048]]),
                    y1[:, sl * 2048:(sl + 1) * 2048],
                )
    nc.compile()
    return nc


def kernel(**inputs):
    xyzl = np.asarray(inputs["point_xyz_large"], dtype=np.float32)
    xyzf = np.asarray(inputs["point_xyz_few"], dtype=np.float32)
    featf = np.asarray(inputs["point_feature_few"], dtype=np.float32)
    featl = np.asarray(inputs["point_feature_large"], dtype=np.float32)
    w0 = np.asarray(inputs["W0"], dtype=np.float32)
    w1 = np.asarray(inputs["W1"], dtype=np.float32)
    g0 = np.asarray(inputs["g0"], dtype=np.float32)
    bt0 = np.asarray(inputs["beta0"], dtype=np.float32)
    g1 = np.asarray(inputs["g1"], dtype=np.float32)
    bt1 = np.asarray(inputs["beta1"], dtype=np.float32)

    w0t = np.ascontiguousarray(w0.T.astype(BF16NP))   # [384, 256]
    w1t = np.ascontiguousarray(w1.T.astype(BF16NP))   # [256, 128]

    if "rep" not in _CACHE:
        rep = np.zeros((8, 128, 128), np.float16)
        for c in range(8):
            for m in range(128):
                rep[c, 16 * c + m % 16, m] = 1.0
        _CACHE["rep"] = rep
    rep = _CACHE["rep"]

    if "nc" not in _CACHE:
        _CACHE["nc"] = _build()
    nc = _CACHE["nc"]

    in_maps = []
    for b in range(B):
        m = _pack_core(xyzl[b], xyzf[b], featf[b], featl[b])
        m.update({
            "rep": rep, "w0t": w0t, "w1t": w1t,
            "g0": g0, "bt0": bt0, "g1": g1, "bt1": bt1,
        })
        in_maps.append(m)

    res = run_bass_kernel_spmd(
        nc, in_maps, list(range(B)), trace=_CACHE.get("trace", False)
    )
    out = np.stack([np.asarray(res.results[b]["out"]) for b in range(B)], 0)
    _CACHE["last_res"] = res
    return out.astype(np.float32)
